# revision 50
# baseline (speedup 1.0000x reference)
"""Trainium2 Bass kernel for nn_Discriminator (minibatch-discrimination GAN critic).

Sharding: data-parallel over batch N=4096 across 8 NeuronCores (512 rows each).
The batch-diversity pairwise term needs the full-batch t = h@wb+bb activation
(only 4096x15): t is AllGathered in fp16 each layer, then every core computes
div for its own 512 rows against all 4096 columns.

Layout: dense chain is feature-major (h^T), so given weights serve directly as
matmul lhsT and per-feature biases are per-partition ScalarE bias operands.

Pairwise inner loop per (kernel k, i-tile): three fused |R_d - t_i| via
tensor_scalar(subtract, abs_max 0) in fp16 (4x DVE mode), two tensor_tensor
adds, then one ScalarE Exp(scale=-1) whose accum_out performs the j-sum.
"""

import sys
import numpy as np

sys.path.insert(0, "/opt/trn_rl_repo")

N = 4096
N_CORES = 8
N_LOC = N // N_CORES          # 512 rows per core
N_FEAT = 512
HID = 1024
KK = 5                        # N_KERNELS
DD = 3                        # KERNEL_DIM
KD = KK * DD                  # 15
DCAT = HID + KK               # 1029
DCAT_PAD = 1152               # 9 * 128
EPS = 1e-3
ALPHA = 0.3
NB_H = HID // 128             # 8 hidden-feature blocks
NB_CAT = 9                    # 8 full + 1 partial (5 rows)
IT = N_LOC // 128             # 4 i-tiles per core

_cache = {}
import os
BUFS = {k: int(v) for k, v in (p.split('=') for p in os.environ.get('KBUFS', '').split(',') if p)}



def _build(debug=False, solo=False, stub_pairwise=False):
    import concourse.bass as bass
    import concourse.bacc as bacc
    import concourse.mybir as mybir
    import concourse.tile as tile
    from concourse import masks

    f32 = mybir.dt.float32
    f16 = mybir.dt.float16
    AF = mybir.ActivationFunctionType
    OP = mybir.AluOpType

    nc = bacc.Bacc("TRN2", target_bir_lowering=False, debug=False,
                   num_devices=1 if solo else N_CORES)

    def din(name, shape, dt=f32):
        return nc.dram_tensor(name, shape, dt, kind="ExternalInput").ap()

    xT = din("xT", (N_FEAT, N_LOC), f16)       # this core's x rows, transposed
    w0a = din("w0a", (N_FEAT, HID), f16)
    b0a = din("b0a", (HID,))
    w0b = din("w0b", (HID, KD), f16)
    b0b = din("b0b", (KD,))
    b0b_r = din("b0b_r", (KD,), f16)
    beta0 = din("beta0", (DCAT_PAD,))
    beta0s = din("beta0s", (DCAT_PAD,))        # ALPHA*beta0
    w1a = din("w1a", (DCAT_PAD, HID), f16)     # zero-padded rows
    b1a = din("b1a", (HID,))
    w1b = din("w1b", (HID, KD), f16)
    b1b = din("b1b", (KD,))
    b1b_r = din("b1b_r", (KD,), f16)
    beta1 = din("beta1", (DCAT_PAD,))
    beta1s = din("beta1s", (DCAT_PAD,))
    wf = din("wf", (DCAT_PAD,), f16)
    bf = din("bf", (1,))
    tg_in = None
    if solo:
        tg_in = [din("tg0", (N_CORES, KD, N_LOC), f16),
                 din("tg1", (N_CORES, KD, N_LOC), f16)]
    y = nc.dram_tensor("y", (N_LOC, 1), f32, kind="ExternalOutput").ap()
    dbg = {}
    if debug:
        for nm, shape, dt in [
            ("dbg_h0", (128, N_LOC), f16),
            ("dbg_tT0", (KD, N_LOC), f16),
            ("dbg_tloc0", (128, KD), f32),
            ("dbg_div0", (128, KK), f32),
            ("dbg_divT0", (KK, N_LOC), f16),
            ("dbg_cat0", (128, N_LOC), f16),
            ("dbg_cat8", (128, N_LOC), f16),
            ("dbg_h1", (128, N_LOC), f16),
            ("dbg_tT1", (KD, N_LOC), f16),
            ("dbg_div1", (128, KK), f32),
            ("dbg_c1_0", (128, N_LOC), f16),
            ("dbg_c1_8", (128, N_LOC), f16),
            ("dbg_s1_1", (1, N_LOC), f32),
            ("dbg_s2_1", (1, N_LOC), f32),
            ("dbg_mu_1", (1, N_LOC), f32),
            ("dbg_vare_1", (1, N_LOC), f32),
            ("dbg_r_1", (1, N_LOC), f32),
        ]:
            dbg[nm] = nc.dram_tensor(nm, shape, dt, kind="ExternalOutput").ap()

    with tile.TileContext(nc) as tc:
        with (
            tc.tile_pool(name="const", bufs=1) as cp,
            tc.tile_pool(name="acts", bufs=1) as ap_,
            tc.tile_pool(name="wa", bufs=1) as wp,
            tc.tile_pool(name="pw_a", bufs=BUFS.get("pa", 4)) as pa,
            tc.tile_pool(name="pw_s", bufs=BUFS.get("ps", 3)) as psb,
            tc.tile_pool(name="pw_e", bufs=BUFS.get("pe", 1)) as pe_,
            tc.tile_pool(name="sq", bufs=2) as sqp,
            tc.tile_pool(name="rows", bufs=1) as rp,
            tc.tile_pool(name="R", bufs=2) as Rp,
            tc.tile_pool(name="psum", bufs=2, space="PSUM") as pp,
            tc.tile_pool(name="psum_b", bufs=1, space="PSUM") as ppb,
            tc.tile_pool(name="psum_s", bufs=2, space="PSUM") as pps,
            tc.tile_pool(name="psum_ln", bufs=1, space="PSUM") as ppl,
            tc.tile_pool(name="dram", bufs=2, space="DRAM") as dp,
        ):
            # ---------------- constants / weights ----------------
            ones_col16 = cp.tile([128, 1], f16, tag="ones_col16")
            nc.vector.memset(ones_col16[:], 1.0)
            ones_col32 = cp.tile([128, 1], f32, tag="ones_col32")
            nc.vector.memset(ones_col32[:], 1.0)
            ones_row16 = cp.tile([1, 128], f16, tag="ones_row16")
            nc.vector.memset(ones_row16[:], 1.0)
            ones_row32 = cp.tile([1, 128], f32, tag="ones_row32")
            nc.vector.memset(ones_row32[:], 1.0)
            ident = cp.tile([128, 128], f32, tag="ident")
            masks.make_identity(nc, ident[:])

            xT_sb = []
            for b in range(N_FEAT // 128):
                t = wp.tile([128, HID], f16, tag=f"wa1_{b+4}", name=f"xT{b}")[:, 0:N_LOC]
                nc.sync.dma_start(t[:], xT[b * 128:(b + 1) * 128, :])
                xT_sb.append(t)

            # w0a k-tiles share slots with w1a k-tiles (w0a dead after layer 0)
            w0a_sb = []
            for kt in range(4):
                t = wp.tile([128, HID], f16, tag=f"wa{kt}")
                nc.sync.dma_start(t[:], w0a[kt * 128:(kt + 1) * 128, :])
                w0a_sb.append(t)

            w0b_sb = []
            w1b_sb = []
            for kt in range(NB_H):
                t = cp.tile([128, KD], f16, tag=f"w0b{kt}")
                nc.sync.dma_start(t[:], w0b[kt * 128:(kt + 1) * 128, :])
                w0b_sb.append(t)
                t = cp.tile([128, KD], f16, tag=f"w1b{kt}")
                nc.sync.dma_start(t[:], w1b[kt * 128:(kt + 1) * 128, :])
                w1b_sb.append(t)

            def load_vec_blocks(ap, n, tag, dt=f32):
                # [n*128] dram vector -> SBUF [128, n]
                t = cp.tile([128, n], dt, tag=tag)
                nc.sync.dma_start(t[:], ap.rearrange("(a b) -> b a", b=128))
                return t

            b0a_sb = load_vec_blocks(b0a, NB_H, "b0a")
            b1a_sb = load_vec_blocks(b1a, NB_H, "b1a")
            beta_sb = [load_vec_blocks(beta0, NB_CAT, "beta0"),
                       load_vec_blocks(beta1, NB_CAT, "beta1")]
            betas_sb = [load_vec_blocks(beta0s, NB_CAT, "beta0s"),
                        load_vec_blocks(beta1s, NB_CAT, "beta1s")]
            wf_sb = load_vec_blocks(wf, NB_CAT, "wf", f16)

            b0b_col = cp.tile([KD, 1], f32, tag="b0b_col")
            nc.sync.dma_start(b0b_col[:], b0b.unsqueeze(1))
            b0b_row = cp.tile([1, KD], f16, tag="b0b_row")
            nc.sync.dma_start(b0b_row[:], b0b_r.unsqueeze(0))
            b1b_col = cp.tile([KD, 1], f32, tag="b1b_col")
            nc.sync.dma_start(b1b_col[:], b1b.unsqueeze(1))
            b1b_row = cp.tile([1, KD], f16, tag="b1b_row")
            nc.sync.dma_start(b1b_row[:], b1b_r.unsqueeze(0))
            bf_sb = cp.tile([1, 1], f32, tag="bf")
            nc.sync.dma_start(bf_sb[:], bf.unsqueeze(0))

            def dense(rhs_blocks, wa_tiles, nkt, ba_col_tile):
                """out^T[f, i] = wa.T @ rhs + per-feature bias; 8 f16 blocks."""
                out = []
                for ob in range(NB_H):
                    ps = pp.tile([128, N_LOC], f32, tag="dense_ps")
                    for kt in range(nkt):
                        nc.tensor.matmul(
                            ps[:],
                            wa_tiles[kt][:, ob * 128:(ob + 1) * 128],
                            rhs_blocks[kt][:],
                            start=(kt == 0), stop=(kt == nkt - 1),
                        )
                    hb = ap_.tile([128, N_LOC], f16, tag=f"h_{ob}")
                    nc.scalar.activation(hb[:], ps[:], AF.Identity,
                                         bias=ba_col_tile[:, ob:ob + 1])
                    out.append(hb)
                return out

            def t_paths(h_blocks, wb_tiles, bb_col, bb_row):
                nt_loc = []
                # t^T [15, N_LOC] fp16 for the gather; t_loc [128,15] fp16 x IT
                ps = pps.tile([KD, N_LOC], f32, tag="small_ps")
                for kt in range(NB_H):
                    nc.tensor.matmul(ps[:], wb_tiles[kt][:], h_blocks[kt][:],
                                     start=(kt == 0), stop=(kt == NB_H - 1))
                tT = ap_.tile([KD, N_LOC], f16, tag="tT")
                nc.scalar.activation(tT[:], ps[:], AF.Identity, bias=bb_col[:])
                t_loc = []
                for it in range(IT):
                    psl = pps.tile([128, KD], f32, tag="small_ps")
                    for kt in range(NB_H):
                        nc.tensor.matmul(
                            psl[:],
                            h_blocks[kt][:, it * 128:(it + 1) * 128],
                            wb_tiles[kt][:],
                            start=(kt == 0), stop=False,
                        )
                    nc.tensor.matmul(psl[:], ones_row16[:], bb_row[:],
                                     start=False, stop=True)
                    tl16 = ap_.tile([128, KD], f16, tag=f"tloc16_{it}")
                    nc.scalar.activation(tl16[:], psl[:], AF.Copy)
                    # fp32 view of the fp16-rounded values (scalar operand must
                    # be fp32; matching the gathered row quantization keeps the
                    # pairwise diagonal exactly zero)
                    tl = ap_.tile([128, KD], f32, tag=f"tloc{it}")
                    nc.vector.tensor_copy(tl[:], tl16[:])
                    t_loc.append(tl)
                    ntl = ap_.tile([128, KD], f32, tag=f"ntloc{it}")
                    nc.vector.tensor_scalar_mul(ntl[:], tl16[:], -1.0)
                    nt_loc.append(ntl)
                return tT, t_loc, nt_loc

            def gather_t(tT, layer):
                if solo:
                    return tg_in[layer]
                inb = dp.tile([KD, N_LOC], f16, tag="cc_in")
                outb = dp.tile([N_CORES, KD, N_LOC], f16, tag="cc_out")
                nc.sync.dma_start(inb[:], tT[:])
                nc.gpsimd.collective_compute(
                    "AllGather",
                    OP.bypass,
                    replica_groups=[list(range(N_CORES))],
                    ins=[inb.opt()],
                    outs=[outb.opt()],
                )
                return outb

            def pairwise(outb, t_loc, nt_loc):
                """div_sb[it] [128, KK] f32: sum_j exp(-sum_d |t_i - t_j|)."""
                div_sb = [ap_.tile([128, KK], f32, tag=f"div{it}",
                                   name=f"div{it}")
                          for it in range(IT)]
                if stub_pairwise:
                    for it in range(IT):
                        nc.vector.memset(div_sb[it][:], 1.0)
                    return div_sb
                for k in range(KK):
                    Rk = Rp.tile([128, DD, N], f16, tag="Rk", name="Rk", bufs=BUFS.get("rk", 2))
                    for d in range(DD):
                        src_ = (outb[:, k * DD + d, :]
                                .unsqueeze(0).partition_broadcast(128))
                        nc.sync.dma_start(
                            Rk[:, d, :].rearrange("p (c j) -> p c j",
                                                  c=N_CORES),
                            src_)
                    for it in range(IT):
                        # ~1/3 of tiles compute |R - t_i| fully on ScalarE
                        # (Abs with bias=-t_i); the rest on DVE via sub then
                        # fp16 sign-bit clear (both 4x-mode tensor_scalar;
                        # scalar_tensor_tensor would be 1x)
                        idx = k * IT + it
                        use_act = idx % 3 == 0
                        aa = []
                        for d in range(DD):
                            kd = k * DD + d
                            a = pa.tile([128, N], f16, tag="pw_a",
                                        name="pw_a")
                            if use_act:
                                nc.scalar.activation(
                                    a[:], Rk[:, d, :], AF.Abs,
                                    bias=nt_loc[it][:, kd:kd + 1])
                            else:
                                tcol = t_loc[it][:, kd:kd + 1]
                                dd_ = pa.tile([128, N], f16, tag="pw_n",
                                              bufs=BUFS.get("pn", 3),
                                              name="dd")
                                nc.vector.tensor_scalar(
                                    dd_[:], Rk[:, d, :], tcol, None,
                                    op0=OP.subtract)
                                nc.vector.tensor_scalar(
                                    a[:].bitcast(mybir.dt.uint16),
                                    dd_[:].bitcast(mybir.dt.uint16),
                                    0x7FFF, None, op0=OP.bitwise_and)
                            aa.append(a)
                        s01 = psb.tile([128, N], f16, tag="pw_s", name="s01")
                        nc.vector.tensor_add(s01[:], aa[0][:], aa[1][:])
                        s = psb.tile([128, N], f16, tag="pw_s", name="s")
                        nc.vector.tensor_add(s[:], s01[:], aa[2][:])
                        e = pe_.tile([128, N], f16, tag="pw_e", name="e")
                        nc.scalar.activation(
                            e[:], s[:], AF.Exp, scale=-1.0,
                            accum_out=div_sb[it][:, k:k + 1])
                return div_sb

            def div_transpose(div_sb):
                # div_sb (IT x [128, KK] f32) -> divT [KK, N_LOC] f16
                divT = ap_.tile([KK, N_LOC], f16, tag="divT")
                for it in range(IT):
                    ps = pps.tile([KK, 128], f32, tag="small_ps")
                    nc.tensor.transpose(ps[:], div_sb[it][:], ident[:])
                    nc.scalar.activation(divT[:, it * 128:(it + 1) * 128],
                                         ps[:], AF.Copy)
                return divT

            def layernorm_leaky(h_blocks, divT, layer):
                """leaky(LN_center(cat(h, div)) + beta); returns 9 f16 blocks."""
                blocks = [(hb, 128) for hb in h_blocks] + [(divT, KK)]
                ps1 = ppl.tile([1, N_LOC], f32, tag="ln_s1")
                ps2 = ppl.tile([1, N_LOC], f32, tag="ln_s2")
                nblk = len(blocks)
                for bi, (blk, nr) in enumerate(blocks):
                    nc.tensor.matmul(ps1[:], ones_col16[0:nr, :], blk[0:nr, :],
                                     start=(bi == 0), stop=(bi == nblk - 1))
                for bi, (blk, nr) in enumerate(blocks):
                    sq = sqp.tile([128, N_LOC], f32, tag="sq")
                    nc.scalar.activation(sq[0:nr, :], blk[0:nr, :], AF.Square)
                    nc.tensor.matmul(ps2[:], ones_col32[0:nr, :], sq[0:nr, :],
                                     start=(bi == 0), stop=(bi == nblk - 1))
                mu = pa.tile([1, N_LOC], f32, tag="pw_a", name="mu")
                nc.vector.tensor_scalar_mul(mu[:], ps1[:], 1.0 / DCAT)
                m2 = pa.tile([1, N_LOC], f32, tag="pw_a", name="m2")
                nc.vector.tensor_scalar_mul(m2[:], ps2[:], 1.0 / DCAT)
                musq = pa.tile([1, N_LOC], f32, tag="pw_a", name="musq")
                nc.vector.tensor_mul(musq[:], mu[:], mu[:])
                # vare = (m2 + EPS) - mu^2
                vare = pa.tile([1, N_LOC], f32, tag="pw_a", name="vare")
                nc.vector.scalar_tensor_tensor(
                    vare[:], m2[:], EPS, musq[:],
                    op0=OP.add, op1=OP.subtract)
                # rsqrt on DVE (bit-trick + 3 Newton steps) so no ACT
                # table swap is needed (Sqrt/Ln live outside the exp set)
                i32 = mybir.dt.int32
                yh = psb.tile([1, N_LOC], f32, tag="pw_s", name="yh")
                nc.vector.tensor_scalar(yh[:].bitcast(i32),
                                        vare[:].bitcast(i32), 1, None,
                                        op0=OP.arith_shift_right)
                y0 = pa.tile([1, N_LOC], f32, tag="pw_a", name="y0")
                nc.vector.tensor_scalar(y0[:].bitcast(i32),
                                        yh[:].bitcast(i32), 0x5F3759DF, -1,
                                        op0=OP.subtract, op1=OP.mult)
                rrow = y0
                for _ in range(3):
                    ysq = psb.tile([1, N_LOC], f32, tag="pw_s", name="ysq")
                    nc.vector.tensor_mul(ysq[:], rrow[:], rrow[:])
                    vy2 = psb.tile([1, N_LOC], f32, tag="pw_s", name="vy2")
                    nc.vector.tensor_mul(vy2[:], ysq[:], vare[:])
                    corr = psb.tile([1, N_LOC], f32, tag="pw_s", name="corr")
                    nc.vector.tensor_scalar(corr[:], vy2[:], -0.5, 1.5,
                                            op0=OP.mult, op1=OP.add)
                    ynew = pa.tile([1, N_LOC], f32, tag="pw_a",
                                   name="ynew")
                    nc.vector.tensor_mul(ynew[:], rrow[:], corr[:])
                    rrow = ynew
                if debug and layer == 1:
                    d1 = pa.tile([1, N_LOC], f32, tag="pw_a", name="d1")
                    nc.scalar.activation(d1[:], ps1[:], AF.Copy)
                    nc.sync.dma_start(dbg["dbg_s1_1"], d1[:])
                    d2 = pa.tile([1, N_LOC], f32, tag="pw_a", name="d2")
                    nc.scalar.activation(d2[:], ps2[:], AF.Copy)
                    nc.sync.dma_start(dbg["dbg_s2_1"], d2[:])
                    nc.sync.dma_start(dbg["dbg_mu_1"], mu[:])
                    nc.sync.dma_start(dbg["dbg_vare_1"], vare[:])
                    nc.sync.dma_start(dbg["dbg_r_1"], rrow[:])
                Bmu = ppb.tile([128, N_LOC], f32, tag="Bmu")
                nc.tensor.matmul(Bmu[:], ones_row32[:], mu[:])
                Br = ppb.tile([128, N_LOC], f32, tag="Br")
                nc.tensor.matmul(Br[:], ones_row32[:], rrow[:])

                out = []
                for bi, (blk, nr) in enumerate(blocks):
                    ob = ap_.tile([128, N_LOC], f16, tag=f"cat_{bi}")
                    if nr < 128:
                        nc.vector.memset(ob[:], 0.0)
                    u = sqp.tile([128, N_LOC], f32, tag="ln_u", bufs=BUFS.get("lu", 2))
                    nc.vector.tensor_sub(u[0:nr, :], blk[0:nr, :], Bmu[0:nr, :])
                    nrm = sqp.tile([128, N_LOC], f32, tag="ln_n")
                    nc.vector.tensor_mul(nrm[0:nr, :], u[0:nr, :], Br[0:nr, :])
                    # leaky(y) = max(y, ALPHA*y), y = nrm + beta
                    bb = sqp.tile([128, N_LOC], f32, tag="ln_b", bufs=BUFS.get("lb", 1))
                    nc.scalar.activation(bb[0:nr, :], nrm[0:nr, :], AF.Identity,
                                         scale=ALPHA,
                                         bias=betas_sb[layer][0:nr, bi:bi + 1])
                    nc.vector.scalar_tensor_tensor(
                        ob[0:nr, :], nrm[0:nr, :],
                        beta_sb[layer][0:nr, bi:bi + 1], bb[0:nr, :],
                        op0=OP.add, op1=OP.max)
                    out.append(ob)
                return out

            # ================= layer 0 =================
            h0 = dense(xT_sb, w0a_sb, 4, b0a_sb)
            tT0, tloc0, ntloc0 = t_paths(h0, w0b_sb, b0b_col, b0b_row)
            outb0 = gather_t(tT0, 0)
            div0 = pairwise(outb0, tloc0, ntloc0)
            divT0 = div_transpose(div0)
            cat0 = layernorm_leaky(h0, divT0, 0)

            # load w1a into the wa slots (w0a dead after layer-0 dense)
            w1a_sb = []
            for kt in range(NB_CAT):
                t = wp.tile([128, HID], f16,
                            tag=f"wa{kt}" if kt < 4 else f"wa1_{kt}")
                nc.sync.dma_start(t[:], w1a[kt * 128:(kt + 1) * 128, :])
                w1a_sb.append(t)

            # ================= layer 1 =================
            h1 = dense(cat0, w1a_sb, NB_CAT, b1a_sb)
            tT1, tloc1, ntloc1 = t_paths(h1, w1b_sb, b1b_col, b1b_row)
            outb1 = gather_t(tT1, 1)
            div1 = pairwise(outb1, tloc1, ntloc1)
            divT1 = div_transpose(div1)
            cat1 = layernorm_leaky(h1, divT1, 1)

            if debug:
                nc.sync.dma_start(dbg["dbg_h0"], h0[0][:])
                nc.sync.dma_start(dbg["dbg_tT0"], tT0[:])
                nc.sync.dma_start(dbg["dbg_tloc0"], tloc0[0][:])
                nc.sync.dma_start(dbg["dbg_div0"], div0[0][:])
                nc.sync.dma_start(dbg["dbg_divT0"], divT0[:])
                nc.sync.dma_start(dbg["dbg_cat0"], cat0[0][:])
                nc.sync.dma_start(dbg["dbg_cat8"], cat0[8][:])
                nc.sync.dma_start(dbg["dbg_h1"], h1[0][:])
                nc.sync.dma_start(dbg["dbg_tT1"], tT1[:])
                nc.sync.dma_start(dbg["dbg_div1"], div1[0][:])
                nc.sync.dma_start(dbg["dbg_c1_0"], cat1[0][:])
                nc.sync.dma_start(dbg["dbg_c1_8"], cat1[8][:])

            # ================= head =================
            psy = pps.tile([1, N_LOC], f32, tag="small_ps")
            for kt in range(NB_CAT):
                nc.tensor.matmul(psy[:], wf_sb[:, kt:kt + 1], cat1[kt][:],
                                 start=(kt == 0), stop=(kt == NB_CAT - 1))
            yrow = pa.tile([1, N_LOC], f32, tag="pw_a", name="yrow")
            nc.scalar.activation(yrow[:], psy[:], AF.Identity, bias=bf_sb[:])
            nc.sync.dma_start(y.rearrange("n o -> o n"), yrow[:])

    nc.compile()
    return nc


def _prep_inputs(inputs):
    x = np.asarray(inputs["x"], dtype=np.float32)

    def f16(a):
        return np.asarray(a, dtype=np.float16)

    def padv(v, dt=np.float32):
        out = np.zeros((DCAT_PAD,), dtype=dt)
        out[:v.shape[0]] = v
        return out

    w1a_pad = np.zeros((DCAT_PAD, HID), dtype=np.float16)
    w1a_pad[:DCAT] = np.asarray(inputs["w1_a"], dtype=np.float16)
    beta0 = np.asarray(inputs["beta0"], dtype=np.float32)
    beta1 = np.asarray(inputs["beta1"], dtype=np.float32)
    shared = {
        "w0a": f16(inputs["w0_a"]),
        "b0a": np.asarray(inputs["b0_a"], dtype=np.float32),
        "w0b": f16(inputs["w0_b"]),
        "b0b": np.asarray(inputs["b0_b"], dtype=np.float32),
        "b0b_r": f16(inputs["b0_b"]),
        "beta0": padv(beta0), "beta0s": padv(ALPHA * beta0),
        "w1a": w1a_pad,
        "b1a": np.asarray(inputs["b1_a"], dtype=np.float32),
        "w1b": f16(inputs["w1_b"]),
        "b1b": np.asarray(inputs["b1_b"], dtype=np.float32),
        "b1b_r": f16(inputs["b1_b"]),
        "beta1": padv(beta1), "beta1s": padv(ALPHA * beta1),
        "wf": padv(np.asarray(inputs["wf"], dtype=np.float16)[:, 0], np.float16),
        "bf": np.asarray(inputs["bf"], dtype=np.float32),
    }
    in_maps = []
    for c in range(N_CORES):
        m = dict(shared)
        m["xT"] = np.ascontiguousarray(
            x[c * N_LOC:(c + 1) * N_LOC, :].T).astype(np.float16)
        in_maps.append(m)
    return in_maps


def kernel(**inputs):
    from concourse import bass_utils

    if "nc" not in _cache:
        _cache["nc"] = _build()
    nc = _cache["nc"]

    in_maps = _prep_inputs(inputs)
    res = bass_utils.run_bass_kernel_spmd(
        nc, in_maps, core_ids=list(range(N_CORES)))
    y = np.zeros((N, 1), dtype=np.float32)
    for c in range(N_CORES):
        y[c * N_LOC:(c + 1) * N_LOC] = res.results[c]["y"]
    return y


# revision 51
# speedup vs baseline: 1208.6224x; 1208.6224x over previous
"""Trainium2 Bass kernel for nn_Discriminator (minibatch-discrimination GAN critic).

Sharding: data-parallel over batch N=4096 across 8 NeuronCores (512 rows each).
The batch-diversity pairwise term needs the full-batch t = h@wb+bb activation
(only 4096x15): t is AllGathered in fp16 each layer, then every core computes
div for its own 512 rows against all 4096 columns.

Layout: dense chain is feature-major (h^T), so given weights serve directly as
matmul lhsT and per-feature biases are per-partition ScalarE bias operands.

Pairwise inner loop per (kernel k, i-tile), all fp16 [128 x 4096] tiles:
|R_d - t_i| is computed either fully on ScalarE (Abs activation with
bias=-t_i, ~1/3 of tiles) or on DVE as subtract + sign-bit clear
(bitwise_and 0x7fff on the fp16 bits), both 4x-mode tensor_scalar ops.
Two DVE tensor_tensor adds form s = sum_d |.|, then one ScalarE
Exp(scale=-1) with accum_out performs exp and the j-reduction in one pass.
LayerNorm rsqrt runs on DVE (bit-trick + Newton) so ScalarE never swaps
activation tables (everything stays in the exp set).

Engine busy (cost model, per core): DVE ~390us, ACT ~350us, PE ~75us,
DMA ~105us; end-to-end TimelineSim ~496us.
"""

import sys
import numpy as np

sys.path.insert(0, "/opt/trn_rl_repo")

N = 4096
N_CORES = 8
N_LOC = N // N_CORES          # 512 rows per core
N_FEAT = 512
HID = 1024
KK = 5                        # N_KERNELS
DD = 3                        # KERNEL_DIM
KD = KK * DD                  # 15
DCAT = HID + KK               # 1029
DCAT_PAD = 1152               # 9 * 128
EPS = 1e-3
ALPHA = 0.3
NB_H = HID // 128             # 8 hidden-feature blocks
NB_CAT = 9                    # 8 full + 1 partial (5 rows)
IT = N_LOC // 128             # 4 i-tiles per core

_cache = {}
import os
BUFS = {k: int(v) for k, v in (p.split('=') for p in os.environ.get('KBUFS', '').split(',') if p)}



def _build(debug=False, solo=False, stub_pairwise=False):
    import concourse.bass as bass
    import concourse.bacc as bacc
    import concourse.mybir as mybir
    import concourse.tile as tile
    from concourse import masks

    f32 = mybir.dt.float32
    f16 = mybir.dt.float16
    AF = mybir.ActivationFunctionType
    OP = mybir.AluOpType

    nc = bacc.Bacc("TRN2", target_bir_lowering=False, debug=False,
                   num_devices=1 if solo else N_CORES)

    def din(name, shape, dt=f32):
        return nc.dram_tensor(name, shape, dt, kind="ExternalInput").ap()

    xT = din("xT", (N_FEAT, N_LOC), f16)       # this core's x rows, transposed
    w0a = din("w0a", (N_FEAT, HID), f16)
    b0a = din("b0a", (HID,))
    w0b = din("w0b", (HID, KD), f16)
    b0b = din("b0b", (KD,))
    b0b_r = din("b0b_r", (KD,), f16)
    beta0 = din("beta0", (DCAT_PAD,))
    beta0s = din("beta0s", (DCAT_PAD,))        # ALPHA*beta0
    w1a = din("w1a", (DCAT_PAD, HID), f16)     # zero-padded rows
    b1a = din("b1a", (HID,))
    w1b = din("w1b", (HID, KD), f16)
    b1b = din("b1b", (KD,))
    b1b_r = din("b1b_r", (KD,), f16)
    beta1 = din("beta1", (DCAT_PAD,))
    beta1s = din("beta1s", (DCAT_PAD,))
    wf = din("wf", (DCAT_PAD,), f16)
    bf = din("bf", (1,))
    tg_in = None
    if solo:
        tg_in = [din("tg0", (N_CORES, KD, N_LOC), f16),
                 din("tg1", (N_CORES, KD, N_LOC), f16)]
    y = nc.dram_tensor("y", (N_LOC, 1), f32, kind="ExternalOutput").ap()
    dbg = {}
    if debug:
        for nm, shape, dt in [
            ("dbg_h0", (128, N_LOC), f16),
            ("dbg_tT0", (KD, N_LOC), f16),
            ("dbg_tloc0", (128, KD), f32),
            ("dbg_div0", (128, KK), f32),
            ("dbg_divT0", (KK, N_LOC), f16),
            ("dbg_cat0", (128, N_LOC), f16),
            ("dbg_cat8", (128, N_LOC), f16),
            ("dbg_h1", (128, N_LOC), f16),
            ("dbg_tT1", (KD, N_LOC), f16),
            ("dbg_div1", (128, KK), f32),
            ("dbg_c1_0", (128, N_LOC), f16),
            ("dbg_c1_8", (128, N_LOC), f16),
            ("dbg_s1_1", (1, N_LOC), f32),
            ("dbg_s2_1", (1, N_LOC), f32),
            ("dbg_mu_1", (1, N_LOC), f32),
            ("dbg_vare_1", (1, N_LOC), f32),
            ("dbg_r_1", (1, N_LOC), f32),
        ]:
            dbg[nm] = nc.dram_tensor(nm, shape, dt, kind="ExternalOutput").ap()

    with tile.TileContext(nc) as tc:
        with (
            tc.tile_pool(name="const", bufs=1) as cp,
            tc.tile_pool(name="acts", bufs=1) as ap_,
            tc.tile_pool(name="wa", bufs=1) as wp,
            tc.tile_pool(name="pw_a", bufs=BUFS.get("pa", 4)) as pa,
            tc.tile_pool(name="pw_s", bufs=BUFS.get("ps", 3)) as psb,
            tc.tile_pool(name="pw_e", bufs=BUFS.get("pe", 1)) as pe_,
            tc.tile_pool(name="sq", bufs=2) as sqp,
            tc.tile_pool(name="rows", bufs=1) as rp,
            tc.tile_pool(name="R", bufs=2) as Rp,
            tc.tile_pool(name="psum", bufs=2, space="PSUM") as pp,
            tc.tile_pool(name="psum_b", bufs=1, space="PSUM") as ppb,
            tc.tile_pool(name="psum_s", bufs=2, space="PSUM") as pps,
            tc.tile_pool(name="psum_ln", bufs=1, space="PSUM") as ppl,
            tc.tile_pool(name="dram", bufs=2, space="DRAM") as dp,
        ):
            # ---------------- constants / weights ----------------
            ones_col16 = cp.tile([128, 1], f16, tag="ones_col16")
            nc.vector.memset(ones_col16[:], 1.0)
            ones_col32 = cp.tile([128, 1], f32, tag="ones_col32")
            nc.vector.memset(ones_col32[:], 1.0)
            ones_row16 = cp.tile([1, 128], f16, tag="ones_row16")
            nc.vector.memset(ones_row16[:], 1.0)
            ones_row32 = cp.tile([1, 128], f32, tag="ones_row32")
            nc.vector.memset(ones_row32[:], 1.0)
            ident = cp.tile([128, 128], f32, tag="ident")
            masks.make_identity(nc, ident[:])

            xT_sb = []
            for b in range(N_FEAT // 128):
                t = wp.tile([128, HID], f16, tag=f"wa1_{b+4}", name=f"xT{b}")[:, 0:N_LOC]
                nc.sync.dma_start(t[:], xT[b * 128:(b + 1) * 128, :])
                xT_sb.append(t)

            # w0a k-tiles share slots with w1a k-tiles (w0a dead after layer 0)
            w0a_sb = []
            for kt in range(4):
                t = wp.tile([128, HID], f16, tag=f"wa{kt}")
                nc.sync.dma_start(t[:], w0a[kt * 128:(kt + 1) * 128, :])
                w0a_sb.append(t)

            w0b_sb = []
            w1b_sb = []
            for kt in range(NB_H):
                t = cp.tile([128, KD], f16, tag=f"w0b{kt}")
                nc.sync.dma_start(t[:], w0b[kt * 128:(kt + 1) * 128, :])
                w0b_sb.append(t)
                t = cp.tile([128, KD], f16, tag=f"w1b{kt}")
                nc.sync.dma_start(t[:], w1b[kt * 128:(kt + 1) * 128, :])
                w1b_sb.append(t)

            def load_vec_blocks(ap, n, tag, dt=f32):
                # [n*128] dram vector -> SBUF [128, n]
                t = cp.tile([128, n], dt, tag=tag)
                nc.sync.dma_start(t[:], ap.rearrange("(a b) -> b a", b=128))
                return t

            b0a_sb = load_vec_blocks(b0a, NB_H, "b0a")
            b1a_sb = load_vec_blocks(b1a, NB_H, "b1a")
            beta_sb = [load_vec_blocks(beta0, NB_CAT, "beta0"),
                       load_vec_blocks(beta1, NB_CAT, "beta1")]
            betas_sb = [load_vec_blocks(beta0s, NB_CAT, "beta0s"),
                        load_vec_blocks(beta1s, NB_CAT, "beta1s")]
            wf_sb = load_vec_blocks(wf, NB_CAT, "wf", f16)

            b0b_col = cp.tile([KD, 1], f32, tag="b0b_col")
            nc.sync.dma_start(b0b_col[:], b0b.unsqueeze(1))
            b0b_row = cp.tile([1, KD], f16, tag="b0b_row")
            nc.sync.dma_start(b0b_row[:], b0b_r.unsqueeze(0))
            b1b_col = cp.tile([KD, 1], f32, tag="b1b_col")
            nc.sync.dma_start(b1b_col[:], b1b.unsqueeze(1))
            b1b_row = cp.tile([1, KD], f16, tag="b1b_row")
            nc.sync.dma_start(b1b_row[:], b1b_r.unsqueeze(0))
            bf_sb = cp.tile([1, 1], f32, tag="bf")
            nc.sync.dma_start(bf_sb[:], bf.unsqueeze(0))

            def dense(rhs_blocks, wa_tiles, nkt, ba_col_tile):
                """out^T[f, i] = wa.T @ rhs + per-feature bias; 8 f16 blocks."""
                out = []
                for ob in range(NB_H):
                    ps = pp.tile([128, N_LOC], f32, tag="dense_ps")
                    for kt in range(nkt):
                        nc.tensor.matmul(
                            ps[:],
                            wa_tiles[kt][:, ob * 128:(ob + 1) * 128],
                            rhs_blocks[kt][:],
                            start=(kt == 0), stop=(kt == nkt - 1),
                        )
                    hb = ap_.tile([128, N_LOC], f16, tag=f"h_{ob}")
                    nc.scalar.activation(hb[:], ps[:], AF.Identity,
                                         bias=ba_col_tile[:, ob:ob + 1])
                    out.append(hb)
                return out

            def t_paths(h_blocks, wb_tiles, bb_col, bb_row):
                nt_loc = []
                # t^T [15, N_LOC] fp16 for the gather; t_loc [128,15] fp16 x IT
                ps = pps.tile([KD, N_LOC], f32, tag="small_ps")
                for kt in range(NB_H):
                    nc.tensor.matmul(ps[:], wb_tiles[kt][:], h_blocks[kt][:],
                                     start=(kt == 0), stop=(kt == NB_H - 1))
                tT = ap_.tile([KD, N_LOC], f16, tag="tT")
                nc.scalar.activation(tT[:], ps[:], AF.Identity, bias=bb_col[:])
                t_loc = []
                for it in range(IT):
                    psl = pps.tile([128, KD], f32, tag="small_ps")
                    for kt in range(NB_H):
                        nc.tensor.matmul(
                            psl[:],
                            h_blocks[kt][:, it * 128:(it + 1) * 128],
                            wb_tiles[kt][:],
                            start=(kt == 0), stop=False,
                        )
                    nc.tensor.matmul(psl[:], ones_row16[:], bb_row[:],
                                     start=False, stop=True)
                    tl16 = ap_.tile([128, KD], f16, tag=f"tloc16_{it}")
                    nc.scalar.activation(tl16[:], psl[:], AF.Copy)
                    # fp32 view of the fp16-rounded values (scalar operand must
                    # be fp32; matching the gathered row quantization keeps the
                    # pairwise diagonal exactly zero)
                    tl = ap_.tile([128, KD], f32, tag=f"tloc{it}")
                    nc.vector.tensor_copy(tl[:], tl16[:])
                    t_loc.append(tl)
                    ntl = ap_.tile([128, KD], f32, tag=f"ntloc{it}")
                    nc.vector.tensor_scalar_mul(ntl[:], tl16[:], -1.0)
                    nt_loc.append(ntl)
                return tT, t_loc, nt_loc

            def gather_t(tT, layer):
                if solo:
                    return tg_in[layer]
                inb = dp.tile([KD, N_LOC], f16, tag="cc_in")
                outb = dp.tile([N_CORES, KD, N_LOC], f16, tag="cc_out")
                nc.sync.dma_start(inb[:], tT[:])
                nc.gpsimd.collective_compute(
                    "AllGather",
                    OP.bypass,
                    replica_groups=[list(range(N_CORES))],
                    ins=[inb.opt()],
                    outs=[outb.opt()],
                )
                return outb

            def pairwise(outb, t_loc, nt_loc):
                """div_sb[it] [128, KK] f32: sum_j exp(-sum_d |t_i - t_j|)."""
                div_sb = [ap_.tile([128, KK], f32, tag=f"div{it}",
                                   name=f"div{it}")
                          for it in range(IT)]
                if stub_pairwise:
                    for it in range(IT):
                        nc.vector.memset(div_sb[it][:], 1.0)
                    return div_sb
                for k in range(KK):
                    Rk = Rp.tile([128, DD, N], f16, tag="Rk", name="Rk", bufs=BUFS.get("rk", 2))
                    for d in range(DD):
                        src_ = (outb[:, k * DD + d, :]
                                .unsqueeze(0).partition_broadcast(128))
                        nc.sync.dma_start(
                            Rk[:, d, :].rearrange("p (c j) -> p c j",
                                                  c=N_CORES),
                            src_)
                    for it in range(IT):
                        # ~1/3 of tiles compute |R - t_i| fully on ScalarE
                        # (Abs with bias=-t_i); the rest on DVE via sub then
                        # fp16 sign-bit clear (both 4x-mode tensor_scalar;
                        # scalar_tensor_tensor would be 1x)
                        idx = k * IT + it
                        use_act = idx % 3 == 0
                        aa = []
                        for d in range(DD):
                            kd = k * DD + d
                            a = pa.tile([128, N], f16, tag="pw_a",
                                        name="pw_a")
                            if use_act:
                                nc.scalar.activation(
                                    a[:], Rk[:, d, :], AF.Abs,
                                    bias=nt_loc[it][:, kd:kd + 1])
                            else:
                                tcol = t_loc[it][:, kd:kd + 1]
                                dd_ = pa.tile([128, N], f16, tag="pw_n",
                                              bufs=BUFS.get("pn", 3),
                                              name="dd")
                                nc.vector.tensor_scalar(
                                    dd_[:], Rk[:, d, :], tcol, None,
                                    op0=OP.subtract)
                                nc.vector.tensor_scalar(
                                    a[:].bitcast(mybir.dt.uint16),
                                    dd_[:].bitcast(mybir.dt.uint16),
                                    0x7FFF, None, op0=OP.bitwise_and)
                            aa.append(a)
                        s01 = psb.tile([128, N], f16, tag="pw_s", name="s01")
                        nc.vector.tensor_add(s01[:], aa[0][:], aa[1][:])
                        s = psb.tile([128, N], f16, tag="pw_s", name="s")
                        nc.vector.tensor_add(s[:], s01[:], aa[2][:])
                        e = pe_.tile([128, N], f16, tag="pw_e", name="e")
                        nc.scalar.activation(
                            e[:], s[:], AF.Exp, scale=-1.0,
                            accum_out=div_sb[it][:, k:k + 1])
                return div_sb

            def div_transpose(div_sb):
                # div_sb (IT x [128, KK] f32) -> divT [KK, N_LOC] f16
                divT = ap_.tile([KK, N_LOC], f16, tag="divT")
                for it in range(IT):
                    ps = pps.tile([KK, 128], f32, tag="small_ps")
                    nc.tensor.transpose(ps[:], div_sb[it][:], ident[:])
                    nc.scalar.activation(divT[:, it * 128:(it + 1) * 128],
                                         ps[:], AF.Copy)
                return divT

            def layernorm_leaky(h_blocks, divT, layer):
                """leaky(LN_center(cat(h, div)) + beta); returns 9 f16 blocks."""
                blocks = [(hb, 128) for hb in h_blocks] + [(divT, KK)]
                ps1 = ppl.tile([1, N_LOC], f32, tag="ln_s1")
                ps2 = ppl.tile([1, N_LOC], f32, tag="ln_s2")
                nblk = len(blocks)
                for bi, (blk, nr) in enumerate(blocks):
                    nc.tensor.matmul(ps1[:], ones_col16[0:nr, :], blk[0:nr, :],
                                     start=(bi == 0), stop=(bi == nblk - 1))
                for bi, (blk, nr) in enumerate(blocks):
                    sq = sqp.tile([128, N_LOC], f32, tag="sq")
                    nc.scalar.activation(sq[0:nr, :], blk[0:nr, :], AF.Square)
                    nc.tensor.matmul(ps2[:], ones_col32[0:nr, :], sq[0:nr, :],
                                     start=(bi == 0), stop=(bi == nblk - 1))
                mu = pa.tile([1, N_LOC], f32, tag="pw_a", name="mu")
                nc.vector.tensor_scalar_mul(mu[:], ps1[:], 1.0 / DCAT)
                m2 = pa.tile([1, N_LOC], f32, tag="pw_a", name="m2")
                nc.vector.tensor_scalar_mul(m2[:], ps2[:], 1.0 / DCAT)
                musq = pa.tile([1, N_LOC], f32, tag="pw_a", name="musq")
                nc.vector.tensor_mul(musq[:], mu[:], mu[:])
                # vare = (m2 + EPS) - mu^2
                vare = pa.tile([1, N_LOC], f32, tag="pw_a", name="vare")
                nc.vector.scalar_tensor_tensor(
                    vare[:], m2[:], EPS, musq[:],
                    op0=OP.add, op1=OP.subtract)
                # rsqrt on DVE (bit-trick + 3 Newton steps) so no ACT
                # table swap is needed (Sqrt/Ln live outside the exp set)
                i32 = mybir.dt.int32
                yh = psb.tile([1, N_LOC], f32, tag="pw_s", name="yh")
                nc.vector.tensor_scalar(yh[:].bitcast(i32),
                                        vare[:].bitcast(i32), 1, None,
                                        op0=OP.arith_shift_right)
                y0 = pa.tile([1, N_LOC], f32, tag="pw_a", name="y0")
                nc.vector.tensor_scalar(y0[:].bitcast(i32),
                                        yh[:].bitcast(i32), 0x5F3759DF, -1,
                                        op0=OP.subtract, op1=OP.mult)
                rrow = y0
                for _ in range(3):
                    ysq = psb.tile([1, N_LOC], f32, tag="pw_s", name="ysq")
                    nc.vector.tensor_mul(ysq[:], rrow[:], rrow[:])
                    vy2 = psb.tile([1, N_LOC], f32, tag="pw_s", name="vy2")
                    nc.vector.tensor_mul(vy2[:], ysq[:], vare[:])
                    corr = psb.tile([1, N_LOC], f32, tag="pw_s", name="corr")
                    nc.vector.tensor_scalar(corr[:], vy2[:], -0.5, 1.5,
                                            op0=OP.mult, op1=OP.add)
                    ynew = pa.tile([1, N_LOC], f32, tag="pw_a",
                                   name="ynew")
                    nc.vector.tensor_mul(ynew[:], rrow[:], corr[:])
                    rrow = ynew
                if debug and layer == 1:
                    d1 = pa.tile([1, N_LOC], f32, tag="pw_a", name="d1")
                    nc.scalar.activation(d1[:], ps1[:], AF.Copy)
                    nc.sync.dma_start(dbg["dbg_s1_1"], d1[:])
                    d2 = pa.tile([1, N_LOC], f32, tag="pw_a", name="d2")
                    nc.scalar.activation(d2[:], ps2[:], AF.Copy)
                    nc.sync.dma_start(dbg["dbg_s2_1"], d2[:])
                    nc.sync.dma_start(dbg["dbg_mu_1"], mu[:])
                    nc.sync.dma_start(dbg["dbg_vare_1"], vare[:])
                    nc.sync.dma_start(dbg["dbg_r_1"], rrow[:])
                Bmu = ppb.tile([128, N_LOC], f32, tag="Bmu")
                nc.tensor.matmul(Bmu[:], ones_row32[:], mu[:])
                Br = ppb.tile([128, N_LOC], f32, tag="Br")
                nc.tensor.matmul(Br[:], ones_row32[:], rrow[:])

                out = []
                for bi, (blk, nr) in enumerate(blocks):
                    ob = ap_.tile([128, N_LOC], f16, tag=f"cat_{bi}")
                    if nr < 128:
                        nc.vector.memset(ob[:], 0.0)
                    u = sqp.tile([128, N_LOC], f32, tag="ln_u", bufs=BUFS.get("lu", 2))
                    nc.vector.tensor_sub(u[0:nr, :], blk[0:nr, :], Bmu[0:nr, :])
                    nrm = sqp.tile([128, N_LOC], f32, tag="ln_n")
                    nc.vector.tensor_mul(nrm[0:nr, :], u[0:nr, :], Br[0:nr, :])
                    # leaky(y) = max(y, ALPHA*y), y = nrm + beta
                    bb = sqp.tile([128, N_LOC], f32, tag="ln_b", bufs=BUFS.get("lb", 1))
                    nc.scalar.activation(bb[0:nr, :], nrm[0:nr, :], AF.Identity,
                                         scale=ALPHA,
                                         bias=betas_sb[layer][0:nr, bi:bi + 1])
                    nc.vector.scalar_tensor_tensor(
                        ob[0:nr, :], nrm[0:nr, :],
                        beta_sb[layer][0:nr, bi:bi + 1], bb[0:nr, :],
                        op0=OP.add, op1=OP.max)
                    out.append(ob)
                return out

            # ================= layer 0 =================
            h0 = dense(xT_sb, w0a_sb, 4, b0a_sb)
            tT0, tloc0, ntloc0 = t_paths(h0, w0b_sb, b0b_col, b0b_row)
            outb0 = gather_t(tT0, 0)
            div0 = pairwise(outb0, tloc0, ntloc0)
            divT0 = div_transpose(div0)
            cat0 = layernorm_leaky(h0, divT0, 0)

            # load w1a into the wa slots (w0a dead after layer-0 dense)
            w1a_sb = []
            for kt in range(NB_CAT):
                t = wp.tile([128, HID], f16,
                            tag=f"wa{kt}" if kt < 4 else f"wa1_{kt}")
                nc.sync.dma_start(t[:], w1a[kt * 128:(kt + 1) * 128, :])
                w1a_sb.append(t)

            # ================= layer 1 =================
            h1 = dense(cat0, w1a_sb, NB_CAT, b1a_sb)
            tT1, tloc1, ntloc1 = t_paths(h1, w1b_sb, b1b_col, b1b_row)
            outb1 = gather_t(tT1, 1)
            div1 = pairwise(outb1, tloc1, ntloc1)
            divT1 = div_transpose(div1)
            cat1 = layernorm_leaky(h1, divT1, 1)

            if debug:
                nc.sync.dma_start(dbg["dbg_h0"], h0[0][:])
                nc.sync.dma_start(dbg["dbg_tT0"], tT0[:])
                nc.sync.dma_start(dbg["dbg_tloc0"], tloc0[0][:])
                nc.sync.dma_start(dbg["dbg_div0"], div0[0][:])
                nc.sync.dma_start(dbg["dbg_divT0"], divT0[:])
                nc.sync.dma_start(dbg["dbg_cat0"], cat0[0][:])
                nc.sync.dma_start(dbg["dbg_cat8"], cat0[8][:])
                nc.sync.dma_start(dbg["dbg_h1"], h1[0][:])
                nc.sync.dma_start(dbg["dbg_tT1"], tT1[:])
                nc.sync.dma_start(dbg["dbg_div1"], div1[0][:])
                nc.sync.dma_start(dbg["dbg_c1_0"], cat1[0][:])
                nc.sync.dma_start(dbg["dbg_c1_8"], cat1[8][:])

            # ================= head =================
            psy = pps.tile([1, N_LOC], f32, tag="small_ps")
            for kt in range(NB_CAT):
                nc.tensor.matmul(psy[:], wf_sb[:, kt:kt + 1], cat1[kt][:],
                                 start=(kt == 0), stop=(kt == NB_CAT - 1))
            yrow = pa.tile([1, N_LOC], f32, tag="pw_a", name="yrow")
            nc.scalar.activation(yrow[:], psy[:], AF.Identity, bias=bf_sb[:])
            nc.sync.dma_start(y.rearrange("n o -> o n"), yrow[:])

    nc.compile()
    return nc


def _prep_inputs(inputs):
    x = np.asarray(inputs["x"], dtype=np.float32)

    def f16(a):
        return np.asarray(a, dtype=np.float16)

    def padv(v, dt=np.float32):
        out = np.zeros((DCAT_PAD,), dtype=dt)
        out[:v.shape[0]] = v
        return out

    w1a_pad = np.zeros((DCAT_PAD, HID), dtype=np.float16)
    w1a_pad[:DCAT] = np.asarray(inputs["w1_a"], dtype=np.float16)
    beta0 = np.asarray(inputs["beta0"], dtype=np.float32)
    beta1 = np.asarray(inputs["beta1"], dtype=np.float32)
    shared = {
        "w0a": f16(inputs["w0_a"]),
        "b0a": np.asarray(inputs["b0_a"], dtype=np.float32),
        "w0b": f16(inputs["w0_b"]),
        "b0b": np.asarray(inputs["b0_b"], dtype=np.float32),
        "b0b_r": f16(inputs["b0_b"]),
        "beta0": padv(beta0), "beta0s": padv(ALPHA * beta0),
        "w1a": w1a_pad,
        "b1a": np.asarray(inputs["b1_a"], dtype=np.float32),
        "w1b": f16(inputs["w1_b"]),
        "b1b": np.asarray(inputs["b1_b"], dtype=np.float32),
        "b1b_r": f16(inputs["b1_b"]),
        "beta1": padv(beta1), "beta1s": padv(ALPHA * beta1),
        "wf": padv(np.asarray(inputs["wf"], dtype=np.float16)[:, 0], np.float16),
        "bf": np.asarray(inputs["bf"], dtype=np.float32),
    }
    in_maps = []
    for c in range(N_CORES):
        m = dict(shared)
        m["xT"] = np.ascontiguousarray(
            x[c * N_LOC:(c + 1) * N_LOC, :].T).astype(np.float16)
        in_maps.append(m)
    return in_maps


def kernel(**inputs):
    from concourse import bass_utils

    if "nc" not in _cache:
        _cache["nc"] = _build()
    nc = _cache["nc"]

    in_maps = _prep_inputs(inputs)
    res = bass_utils.run_bass_kernel_spmd(
        nc, in_maps, core_ids=list(range(N_CORES)))
    y = np.zeros((N, 1), dtype=np.float32)
    for c in range(N_CORES):
        y[c * N_LOC:(c + 1) * N_LOC] = res.results[c]["y"]
    return y


# revision 55
# speedup vs baseline: 1241.8522x; 1.0275x over previous
"""Trainium2 Bass kernel for nn_Discriminator (minibatch-discrimination GAN critic).

Sharding: data-parallel over batch N=4096 across 8 NeuronCores (512 rows each).
The batch-diversity pairwise term needs the full-batch t = h@wb+bb activation
(only 4096x15): t is AllGathered in fp16 each layer, then every core computes
div for its own 512 rows against all 4096 columns.

Layout: dense chain is feature-major (h^T), so given weights serve directly as
matmul lhsT and per-feature biases are per-partition ScalarE bias operands.

Pairwise inner loop per (kernel k, i-tile), all fp16 [128 x 4096] tiles:
|R_d - t_i| is computed either fully on ScalarE (Abs activation with
bias=-t_i, ~1/3 of tiles) or on DVE as subtract + sign-bit clear
(bitwise_and 0x7fff on the fp16 bits), both 4x-mode tensor_scalar ops.
Two DVE tensor_tensor adds form s = sum_d |.|, then one ScalarE
Exp(scale=-1) with accum_out performs exp and the j-reduction in one pass.
LayerNorm rsqrt runs on DVE (bit-trick + Newton) so ScalarE never swaps
activation tables (everything stays in the exp set).

Engine busy (cost model, per core): DVE ~390us, ACT ~350us, PE ~75us,
DMA ~105us; end-to-end TimelineSim ~496us.
"""

import sys
import numpy as np

sys.path.insert(0, "/opt/trn_rl_repo")

N = 4096
N_CORES = 8
N_LOC = N // N_CORES          # 512 rows per core
N_FEAT = 512
HID = 1024
KK = 5                        # N_KERNELS
DD = 3                        # KERNEL_DIM
KD = KK * DD                  # 15
DCAT = HID + KK               # 1029
DCAT_PAD = 1152               # 9 * 128
EPS = 1e-3
ALPHA = 0.3
NB_H = HID // 128             # 8 hidden-feature blocks
NB_CAT = 9                    # 8 full + 1 partial (5 rows)
IT = N_LOC // 128             # 4 i-tiles per core

_cache = {}
import os
BUFS = {k: int(v) for k, v in (p.split('=') for p in os.environ.get('KBUFS', '').split(',') if p)}



def _build(debug=False, solo=False, stub_pairwise=False):
    import concourse.bass as bass
    import concourse.bacc as bacc
    import concourse.mybir as mybir
    import concourse.tile as tile
    from concourse import masks

    f32 = mybir.dt.float32
    f16 = mybir.dt.float16
    AF = mybir.ActivationFunctionType
    OP = mybir.AluOpType

    nc = bacc.Bacc("TRN2", target_bir_lowering=False, debug=False,
                   num_devices=1 if solo else N_CORES)

    def din(name, shape, dt=f32):
        return nc.dram_tensor(name, shape, dt, kind="ExternalInput").ap()

    xT = din("xT", (N_FEAT, N_LOC), f16)       # this core's x rows, transposed
    w0a = din("w0a", (N_FEAT, HID), f16)
    b0a = din("b0a", (HID,))
    w0b = din("w0b", (HID, KD), f16)
    b0b = din("b0b", (KD,))
    b0b_r = din("b0b_r", (KD,), f16)
    beta0 = din("beta0", (DCAT_PAD,))
    beta0s = din("beta0s", (DCAT_PAD,))        # ALPHA*beta0
    w1a = din("w1a", (DCAT_PAD, HID), f16)     # zero-padded rows
    b1a = din("b1a", (HID,))
    w1b = din("w1b", (HID, KD), f16)
    b1b = din("b1b", (KD,))
    b1b_r = din("b1b_r", (KD,), f16)
    beta1 = din("beta1", (DCAT_PAD,))
    beta1s = din("beta1s", (DCAT_PAD,))
    wf = din("wf", (DCAT_PAD,), f16)
    bf = din("bf", (1,))
    tg_in = None
    if solo:
        tg_in = [din("tg0", (N_CORES, KD, N_LOC), f16),
                 din("tg1", (N_CORES, KD, N_LOC), f16)]
    y = nc.dram_tensor("y", (N_LOC, 1), f32, kind="ExternalOutput").ap()
    dbg = {}
    if debug:
        for nm, shape, dt in [
            ("dbg_h0", (128, N_LOC), f16),
            ("dbg_tT0", (KD, N_LOC), f16),
            ("dbg_tloc0", (128, KD), f32),
            ("dbg_div0", (128, KK), f32),
            ("dbg_divT0", (KK, N_LOC), f16),
            ("dbg_cat0", (128, N_LOC), f16),
            ("dbg_cat8", (128, N_LOC), f16),
            ("dbg_h1", (128, N_LOC), f16),
            ("dbg_tT1", (KD, N_LOC), f16),
            ("dbg_div1", (128, KK), f32),
            ("dbg_c1_0", (128, N_LOC), f16),
            ("dbg_c1_8", (128, N_LOC), f16),
            ("dbg_s1_1", (1, N_LOC), f32),
            ("dbg_s2_1", (1, N_LOC), f32),
            ("dbg_mu_1", (1, N_LOC), f32),
            ("dbg_vare_1", (1, N_LOC), f32),
            ("dbg_r_1", (1, N_LOC), f32),
        ]:
            dbg[nm] = nc.dram_tensor(nm, shape, dt, kind="ExternalOutput").ap()

    with tile.TileContext(nc) as tc:
        with (
            tc.tile_pool(name="const", bufs=1) as cp,
            tc.tile_pool(name="acts", bufs=1) as ap_,
            tc.tile_pool(name="wa", bufs=1) as wp,
            tc.tile_pool(name="pw_a", bufs=BUFS.get("pa", 4)) as pa,
            tc.tile_pool(name="pw_s", bufs=BUFS.get("ps", 3)) as psb,
            tc.tile_pool(name="pw_e", bufs=BUFS.get("pe", 1)) as pe_,
            tc.tile_pool(name="sq", bufs=2) as sqp,
            tc.tile_pool(name="rows", bufs=1) as rp,
            tc.tile_pool(name="R", bufs=2) as Rp,
            tc.tile_pool(name="psum", bufs=2, space="PSUM") as pp,
            tc.tile_pool(name="psum_b", bufs=1, space="PSUM") as ppb,
            tc.tile_pool(name="psum_s", bufs=2, space="PSUM") as pps,
            tc.tile_pool(name="psum_ln", bufs=1, space="PSUM") as ppl,
            tc.tile_pool(name="dram", bufs=2, space="DRAM") as dp,
        ):
            # ---------------- constants / weights ----------------
            ones_col16 = cp.tile([128, 1], f16, tag="ones_col16")
            nc.vector.memset(ones_col16[:], 1.0)
            ones_col32 = cp.tile([128, 1], f32, tag="ones_col32")
            nc.vector.memset(ones_col32[:], 1.0)
            ones_row16 = cp.tile([1, 128], f16, tag="ones_row16")
            nc.vector.memset(ones_row16[:], 1.0)
            ones_row32 = cp.tile([1, 128], f32, tag="ones_row32")
            nc.vector.memset(ones_row32[:], 1.0)
            ident = cp.tile([128, 128], f32, tag="ident")
            masks.make_identity(nc, ident[:])

            xT_sb = []
            for b in range(N_FEAT // 128):
                t = wp.tile([128, HID], f16, tag=f"wa1_{b+4}", name=f"xT{b}")[:, 0:N_LOC]
                nc.sync.dma_start(t[:], xT[b * 128:(b + 1) * 128, :])
                xT_sb.append(t)

            # w0a k-tiles share slots with w1a k-tiles (w0a dead after layer 0)
            w0a_sb = []
            for kt in range(4):
                t = wp.tile([128, HID], f16, tag=f"wa{kt}")
                nc.sync.dma_start(t[:], w0a[kt * 128:(kt + 1) * 128, :])
                w0a_sb.append(t)

            w0b_sb = []
            w1b_sb = []
            for kt in range(NB_H):
                t = cp.tile([128, KD], f16, tag=f"w0b{kt}")
                nc.sync.dma_start(t[:], w0b[kt * 128:(kt + 1) * 128, :])
                w0b_sb.append(t)
                t = cp.tile([128, KD], f16, tag=f"w1b{kt}")
                nc.sync.dma_start(t[:], w1b[kt * 128:(kt + 1) * 128, :])
                w1b_sb.append(t)

            def load_vec_blocks(ap, n, tag, dt=f32):
                # [n*128] dram vector -> SBUF [128, n]
                t = cp.tile([128, n], dt, tag=tag)
                nc.sync.dma_start(t[:], ap.rearrange("(a b) -> b a", b=128))
                return t

            b0a_sb = load_vec_blocks(b0a, NB_H, "b0a")
            b1a_sb = load_vec_blocks(b1a, NB_H, "b1a")
            beta_sb = [load_vec_blocks(beta0, NB_CAT, "beta0"),
                       load_vec_blocks(beta1, NB_CAT, "beta1")]
            betas_sb = [load_vec_blocks(beta0s, NB_CAT, "beta0s"),
                        load_vec_blocks(beta1s, NB_CAT, "beta1s")]
            wf_sb = load_vec_blocks(wf, NB_CAT, "wf", f16)

            b0b_col = cp.tile([KD, 1], f32, tag="b0b_col")
            nc.sync.dma_start(b0b_col[:], b0b.unsqueeze(1))
            b0b_row = cp.tile([1, KD], f16, tag="b0b_row")
            nc.sync.dma_start(b0b_row[:], b0b_r.unsqueeze(0))
            b1b_col = cp.tile([KD, 1], f32, tag="b1b_col")
            nc.sync.dma_start(b1b_col[:], b1b.unsqueeze(1))
            b1b_row = cp.tile([1, KD], f16, tag="b1b_row")
            nc.sync.dma_start(b1b_row[:], b1b_r.unsqueeze(0))
            bf_sb = cp.tile([1, 1], f32, tag="bf")
            nc.sync.dma_start(bf_sb[:], bf.unsqueeze(0))

            def dense(rhs_blocks, wa_tiles, nkt, ba_col_tile):
                """out^T[f, i] = wa.T @ rhs + per-feature bias; 8 f16 blocks."""
                out = []
                for ob in range(NB_H):
                    ps = pp.tile([128, N_LOC], f32, tag="dense_ps")
                    for kt in range(nkt):
                        nc.tensor.matmul(
                            ps[:],
                            wa_tiles[kt][:, ob * 128:(ob + 1) * 128],
                            rhs_blocks[kt][:],
                            start=(kt == 0), stop=(kt == nkt - 1),
                        )
                    hb = ap_.tile([128, N_LOC], f16, tag=f"h_{ob}")
                    nc.scalar.activation(hb[:], ps[:], AF.Identity,
                                         bias=ba_col_tile[:, ob:ob + 1])
                    out.append(hb)
                return out

            def t_paths(h_blocks, wb_tiles, bb_col, bb_row):
                nt_loc = []
                # t^T [15, N_LOC] fp16 for the gather; t_loc [128,15] fp16 x IT
                ps = pps.tile([KD, N_LOC], f32, tag="small_ps")
                for kt in range(NB_H):
                    nc.tensor.matmul(ps[:], wb_tiles[kt][:], h_blocks[kt][:],
                                     start=(kt == 0), stop=(kt == NB_H - 1))
                tT = ap_.tile([KD, N_LOC], f16, tag="tT")
                nc.scalar.activation(tT[:], ps[:], AF.Identity, bias=bb_col[:])
                t_loc = []
                for it in range(IT):
                    psl = pps.tile([128, KD], f32, tag="small_ps")
                    for kt in range(NB_H):
                        nc.tensor.matmul(
                            psl[:],
                            h_blocks[kt][:, it * 128:(it + 1) * 128],
                            wb_tiles[kt][:],
                            start=(kt == 0), stop=False,
                        )
                    nc.tensor.matmul(psl[:], ones_row16[:], bb_row[:],
                                     start=False, stop=True)
                    tl16 = ap_.tile([128, KD], f16, tag=f"tloc16_{it}")
                    nc.scalar.activation(tl16[:], psl[:], AF.Copy)
                    # fp32 view of the fp16-rounded values (scalar operand must
                    # be fp32; matching the gathered row quantization keeps the
                    # pairwise diagonal exactly zero)
                    tl = ap_.tile([128, KD], f32, tag=f"tloc{it}")
                    nc.vector.tensor_copy(tl[:], tl16[:])
                    t_loc.append(tl)
                    ntl = ap_.tile([128, KD], f32, tag=f"ntloc{it}")
                    nc.vector.tensor_scalar_mul(ntl[:], tl16[:], -1.0)
                    nt_loc.append(ntl)
                return tT, t_loc, nt_loc

            def gather_t(tT, layer):
                if solo:
                    return tg_in[layer]
                inb = dp.tile([KD, N_LOC], f16, tag="cc_in")
                outb = dp.tile([N_CORES, KD, N_LOC], f16, tag="cc_out")
                nc.sync.dma_start(inb[:], tT[:])
                nc.gpsimd.collective_compute(
                    "AllGather",
                    OP.bypass,
                    replica_groups=[list(range(N_CORES))],
                    ins=[inb.opt()],
                    outs=[outb.opt()],
                )
                return outb

            def pairwise(outb, t_loc, nt_loc):
                """div_sb[it] [128, KK] f32: sum_j exp(-sum_d |t_i - t_j|)."""
                div_sb = [ap_.tile([128, KK], f32, tag=f"div{it}",
                                   name=f"div{it}")
                          for it in range(IT)]
                if stub_pairwise:
                    for it in range(IT):
                        nc.vector.memset(div_sb[it][:], 1.0)
                    return div_sb
                for k in range(KK):
                    Rk = Rp.tile([128, DD, N], f16, tag="Rk", name="Rk", bufs=BUFS.get("rk", 2))
                    for d in range(DD):
                        src_ = (outb[:, k * DD + d, :]
                                .unsqueeze(0).partition_broadcast(128))
                        nc.sync.dma_start(
                            Rk[:, d, :].rearrange("p (c j) -> p c j",
                                                  c=N_CORES),
                            src_)
                    for it in range(IT):
                        # ~1/3 of tiles compute |R - t_i| fully on ScalarE
                        # (Abs with bias=-t_i); the rest on DVE via sub then
                        # fp16 sign-bit clear (both 4x-mode tensor_scalar;
                        # scalar_tensor_tensor would be 1x)
                        idx = k * IT + it
                        aa = []
                        for d in range(DD):
                            kd = k * DD + d
                            use_act = d == 2
                            a = pa.tile([128, N], f16, tag="pw_a",
                                        name="pw_a")
                            if use_act:
                                nc.scalar.activation(
                                    a[:], Rk[:, d, :], AF.Abs,
                                    bias=nt_loc[it][:, kd:kd + 1])
                            else:
                                tcol = t_loc[it][:, kd:kd + 1]
                                dd_ = pa.tile([128, N], f16, tag="pw_n",
                                              bufs=BUFS.get("pn", 3),
                                              name="dd")
                                nc.vector.tensor_scalar(
                                    dd_[:], Rk[:, d, :], tcol, None,
                                    op0=OP.subtract)
                                nc.vector.tensor_scalar(
                                    a[:].bitcast(mybir.dt.uint16),
                                    dd_[:].bitcast(mybir.dt.uint16),
                                    0x7FFF, None, op0=OP.bitwise_and)
                            aa.append(a)
                        s01 = psb.tile([128, N], f16, tag="pw_s", name="s01")
                        nc.vector.tensor_add(s01[:], aa[0][:], aa[1][:])
                        s = psb.tile([128, N], f16, tag="pw_s", name="s")
                        nc.vector.tensor_add(s[:], s01[:], aa[2][:])
                        e = pe_.tile([128, N], f16, tag="pw_e", name="e")
                        nc.scalar.activation(
                            e[:], s[:], AF.Exp, scale=-1.0,
                            accum_out=div_sb[it][:, k:k + 1])
                return div_sb

            def div_transpose(div_sb):
                # div_sb (IT x [128, KK] f32) -> divT [KK, N_LOC] f16
                divT = ap_.tile([KK, N_LOC], f16, tag="divT")
                for it in range(IT):
                    ps = pps.tile([KK, 128], f32, tag="small_ps")
                    nc.tensor.transpose(ps[:], div_sb[it][:], ident[:])
                    nc.scalar.activation(divT[:, it * 128:(it + 1) * 128],
                                         ps[:], AF.Copy)
                return divT

            def layernorm_leaky(h_blocks, divT, layer):
                """leaky(LN_center(cat(h, div)) + beta); returns 9 f16 blocks."""
                blocks = [(hb, 128) for hb in h_blocks] + [(divT, KK)]
                ps1 = ppl.tile([1, N_LOC], f32, tag="ln_s1")
                ps2 = ppl.tile([1, N_LOC], f32, tag="ln_s2")
                nblk = len(blocks)
                for bi, (blk, nr) in enumerate(blocks):
                    nc.tensor.matmul(ps1[:], ones_col16[0:nr, :], blk[0:nr, :],
                                     start=(bi == 0), stop=(bi == nblk - 1))
                for bi, (blk, nr) in enumerate(blocks):
                    sq = sqp.tile([128, N_LOC], f32, tag="sq")
                    nc.scalar.activation(sq[0:nr, :], blk[0:nr, :], AF.Square)
                    nc.tensor.matmul(ps2[:], ones_col32[0:nr, :], sq[0:nr, :],
                                     start=(bi == 0), stop=(bi == nblk - 1))
                mu = pa.tile([1, N_LOC], f32, tag="pw_a", name="mu")
                nc.vector.tensor_scalar_mul(mu[:], ps1[:], 1.0 / DCAT)
                m2 = pa.tile([1, N_LOC], f32, tag="pw_a", name="m2")
                nc.vector.tensor_scalar_mul(m2[:], ps2[:], 1.0 / DCAT)
                musq = pa.tile([1, N_LOC], f32, tag="pw_a", name="musq")
                nc.vector.tensor_mul(musq[:], mu[:], mu[:])
                # vare = (m2 + EPS) - mu^2
                vare = pa.tile([1, N_LOC], f32, tag="pw_a", name="vare")
                nc.vector.scalar_tensor_tensor(
                    vare[:], m2[:], EPS, musq[:],
                    op0=OP.add, op1=OP.subtract)
                # rsqrt on DVE (bit-trick + 3 Newton steps) so no ACT
                # table swap is needed (Sqrt/Ln live outside the exp set)
                i32 = mybir.dt.int32
                yh = psb.tile([1, N_LOC], f32, tag="pw_s", name="yh")
                nc.vector.tensor_scalar(yh[:].bitcast(i32),
                                        vare[:].bitcast(i32), 1, None,
                                        op0=OP.arith_shift_right)
                y0 = pa.tile([1, N_LOC], f32, tag="pw_a", name="y0")
                nc.vector.tensor_scalar(y0[:].bitcast(i32),
                                        yh[:].bitcast(i32), 0x5F3759DF, -1,
                                        op0=OP.subtract, op1=OP.mult)
                rrow = y0
                for _ in range(3):
                    ysq = psb.tile([1, N_LOC], f32, tag="pw_s", name="ysq")
                    nc.vector.tensor_mul(ysq[:], rrow[:], rrow[:])
                    vy2 = psb.tile([1, N_LOC], f32, tag="pw_s", name="vy2")
                    nc.vector.tensor_mul(vy2[:], ysq[:], vare[:])
                    corr = psb.tile([1, N_LOC], f32, tag="pw_s", name="corr")
                    nc.vector.tensor_scalar(corr[:], vy2[:], -0.5, 1.5,
                                            op0=OP.mult, op1=OP.add)
                    ynew = pa.tile([1, N_LOC], f32, tag="pw_a",
                                   name="ynew")
                    nc.vector.tensor_mul(ynew[:], rrow[:], corr[:])
                    rrow = ynew
                if debug and layer == 1:
                    d1 = pa.tile([1, N_LOC], f32, tag="pw_a", name="d1")
                    nc.scalar.activation(d1[:], ps1[:], AF.Copy)
                    nc.sync.dma_start(dbg["dbg_s1_1"], d1[:])
                    d2 = pa.tile([1, N_LOC], f32, tag="pw_a", name="d2")
                    nc.scalar.activation(d2[:], ps2[:], AF.Copy)
                    nc.sync.dma_start(dbg["dbg_s2_1"], d2[:])
                    nc.sync.dma_start(dbg["dbg_mu_1"], mu[:])
                    nc.sync.dma_start(dbg["dbg_vare_1"], vare[:])
                    nc.sync.dma_start(dbg["dbg_r_1"], rrow[:])
                Bmu = ppb.tile([128, N_LOC], f32, tag="Bmu")
                nc.tensor.matmul(Bmu[:], ones_row32[:], mu[:])
                Br = ppb.tile([128, N_LOC], f32, tag="Br")
                nc.tensor.matmul(Br[:], ones_row32[:], rrow[:])

                # fp16 copies of the broadcast rows keep the affine ops in
                # DVE 2x/4x modes (PSUM fp32 operands would force 1x)
                Bmu16 = ap_.tile([128, N_LOC], f16, tag="Bmu16")
                nc.scalar.activation(Bmu16[:], Bmu[:], AF.Copy)
                Br16 = ap_.tile([128, N_LOC], f16, tag="Br16")
                nc.scalar.activation(Br16[:], Br[:], AF.Copy)
                out = []
                for bi, (blk, nr) in enumerate(blocks):
                    ob = ap_.tile([128, N_LOC], f16, tag=f"cat_{bi}")
                    if nr < 128:
                        nc.vector.memset(ob[:], 0.0)
                    u = sqp.tile([128, N_LOC], f16, tag="ln_u", bufs=BUFS.get("lu", 2))
                    nc.vector.tensor_sub(u[0:nr, :], blk[0:nr, :], Bmu16[0:nr, :])
                    nrm = sqp.tile([128, N_LOC], f16, tag="ln_n")
                    nc.vector.tensor_mul(nrm[0:nr, :], u[0:nr, :], Br16[0:nr, :])
                    # leaky(y) = max(y, ALPHA*y), y = nrm + beta
                    bb = sqp.tile([128, N_LOC], f16, tag="ln_b", bufs=BUFS.get("lb", 1))
                    nc.scalar.activation(bb[0:nr, :], nrm[0:nr, :], AF.Identity,
                                         scale=ALPHA,
                                         bias=betas_sb[layer][0:nr, bi:bi + 1])
                    q = sqp.tile([128, N_LOC], f16, tag="ln_q", bufs=2)
                    nc.vector.tensor_scalar(
                        q[0:nr, :], nrm[0:nr, :],
                        beta_sb[layer][0:nr, bi:bi + 1], None, op0=OP.add)
                    nc.vector.tensor_max(ob[0:nr, :], q[0:nr, :], bb[0:nr, :])
                    out.append(ob)
                return out

            # ================= layer 0 =================
            h0 = dense(xT_sb, w0a_sb, 4, b0a_sb)
            tT0, tloc0, ntloc0 = t_paths(h0, w0b_sb, b0b_col, b0b_row)
            outb0 = gather_t(tT0, 0)
            div0 = pairwise(outb0, tloc0, ntloc0)
            divT0 = div_transpose(div0)
            cat0 = layernorm_leaky(h0, divT0, 0)

            # load w1a into the wa slots (w0a dead after layer-0 dense)
            w1a_sb = []
            for kt in range(NB_CAT):
                t = wp.tile([128, HID], f16,
                            tag=f"wa{kt}" if kt < 4 else f"wa1_{kt}")
                nc.sync.dma_start(t[:], w1a[kt * 128:(kt + 1) * 128, :])
                w1a_sb.append(t)

            # ================= layer 1 =================
            h1 = dense(cat0, w1a_sb, NB_CAT, b1a_sb)
            tT1, tloc1, ntloc1 = t_paths(h1, w1b_sb, b1b_col, b1b_row)
            outb1 = gather_t(tT1, 1)
            div1 = pairwise(outb1, tloc1, ntloc1)
            divT1 = div_transpose(div1)
            cat1 = layernorm_leaky(h1, divT1, 1)

            if debug:
                nc.sync.dma_start(dbg["dbg_h0"], h0[0][:])
                nc.sync.dma_start(dbg["dbg_tT0"], tT0[:])
                nc.sync.dma_start(dbg["dbg_tloc0"], tloc0[0][:])
                nc.sync.dma_start(dbg["dbg_div0"], div0[0][:])
                nc.sync.dma_start(dbg["dbg_divT0"], divT0[:])
                nc.sync.dma_start(dbg["dbg_cat0"], cat0[0][:])
                nc.sync.dma_start(dbg["dbg_cat8"], cat0[8][:])
                nc.sync.dma_start(dbg["dbg_h1"], h1[0][:])
                nc.sync.dma_start(dbg["dbg_tT1"], tT1[:])
                nc.sync.dma_start(dbg["dbg_div1"], div1[0][:])
                nc.sync.dma_start(dbg["dbg_c1_0"], cat1[0][:])
                nc.sync.dma_start(dbg["dbg_c1_8"], cat1[8][:])

            # ================= head =================
            psy = pps.tile([1, N_LOC], f32, tag="small_ps")
            for kt in range(NB_CAT):
                nc.tensor.matmul(psy[:], wf_sb[:, kt:kt + 1], cat1[kt][:],
                                 start=(kt == 0), stop=(kt == NB_CAT - 1))
            yrow = pa.tile([1, N_LOC], f32, tag="pw_a", name="yrow")
            nc.scalar.activation(yrow[:], psy[:], AF.Identity, bias=bf_sb[:])
            nc.sync.dma_start(y.rearrange("n o -> o n"), yrow[:])

    nc.compile()
    return nc


def _prep_inputs(inputs):
    x = np.asarray(inputs["x"], dtype=np.float32)

    def f16(a):
        return np.asarray(a, dtype=np.float16)

    def padv(v, dt=np.float32):
        out = np.zeros((DCAT_PAD,), dtype=dt)
        out[:v.shape[0]] = v
        return out

    w1a_pad = np.zeros((DCAT_PAD, HID), dtype=np.float16)
    w1a_pad[:DCAT] = np.asarray(inputs["w1_a"], dtype=np.float16)
    beta0 = np.asarray(inputs["beta0"], dtype=np.float32)
    beta1 = np.asarray(inputs["beta1"], dtype=np.float32)
    shared = {
        "w0a": f16(inputs["w0_a"]),
        "b0a": np.asarray(inputs["b0_a"], dtype=np.float32),
        "w0b": f16(inputs["w0_b"]),
        "b0b": np.asarray(inputs["b0_b"], dtype=np.float32),
        "b0b_r": f16(inputs["b0_b"]),
        "beta0": padv(beta0), "beta0s": padv(ALPHA * beta0),
        "w1a": w1a_pad,
        "b1a": np.asarray(inputs["b1_a"], dtype=np.float32),
        "w1b": f16(inputs["w1_b"]),
        "b1b": np.asarray(inputs["b1_b"], dtype=np.float32),
        "b1b_r": f16(inputs["b1_b"]),
        "beta1": padv(beta1), "beta1s": padv(ALPHA * beta1),
        "wf": padv(np.asarray(inputs["wf"], dtype=np.float16)[:, 0], np.float16),
        "bf": np.asarray(inputs["bf"], dtype=np.float32),
    }
    in_maps = []
    for c in range(N_CORES):
        m = dict(shared)
        m["xT"] = np.ascontiguousarray(
            x[c * N_LOC:(c + 1) * N_LOC, :].T).astype(np.float16)
        in_maps.append(m)
    return in_maps


def kernel(**inputs):
    from concourse import bass_utils

    if "nc" not in _cache:
        _cache["nc"] = _build()
    nc = _cache["nc"]

    in_maps = _prep_inputs(inputs)
    res = bass_utils.run_bass_kernel_spmd(
        nc, in_maps, core_ids=list(range(N_CORES)))
    y = np.zeros((N, 1), dtype=np.float32)
    for c in range(N_CORES):
        y[c * N_LOC:(c + 1) * N_LOC] = res.results[c]["y"]
    return y
